# revision 23
# baseline (speedup 1.0000x reference)
"""Trainium2 Bass kernel: VAE-style AttnBlock.

  y = x + proj( attention( q(gn(x)), k(gn(x)), v(gn(x)) ) )

  x: [2, 512, 64, 64] f32, gn = GroupNorm(8 groups, eps=1e-6),
  q/k/v/proj = 1x1 convs (512x512), attention over the 4096 spatial
  positions with softmax along the key axis, scale = 512**-0.5.

Sharding: 8 cores = (batch b, query-block qb); each core computes the
softmax rows for its 1024 query positions of batch b against the full
K/V of that batch (K/V conv is recomputed per core - cheaper than a
cross-core exchange at this size). Conv weights replicated.

Device-side structure: GroupNorm is folded into the conv weights.
  xn[c,:] = x[c,:]*s_c + t_c   with s_c = rstd_g*norm_w_c,
                                    t_c = norm_b_c - mean_g*s_c
  conv(xn) = (W*s) @ x + (W @ t + b)
After computing group stats on device, the transposed conv weights are
scaled by s per input-channel (cast to bf16), and effective biases are
computed with tiny matmuls (rhs = t/s, against the scaled weights).
The k-bias is skipped: softmax_j((Q0+bq).(K0+bk)) = softmax_j((Q0+bq).K0)
since the bk term only adds a per-row constant. The v-bias (sum of the
softmax weights times a constant = the constant) is folded through the
proj conv into the output bias.

Softmax runs without max-subtraction: logits here are ~N(0,1) after the
1/sqrt(C) scale, so exp stays comfortably finite in fp32.

Matmul dtype is bf16 with fp32 PSUM accumulation throughout (incl. Q@K
and A@V); the softmax normalizer, proj epilogue and residual are fp32.
"""

import numpy as np
import ml_dtypes

import concourse.bacc as bacc
import concourse.tile as tile
from concourse import mybir
from concourse import bass_utils

B, C, H, W = 2, 512, 64, 64
HW = H * W              # 4096 spatial positions
P = 128                 # partitions
KC = C // P             # 4 channel chunks
NCORES = 8
QB = B * HW // NCORES   # 1024 query positions per core
NIH = 2                 # query halves of 512
G = 8                   # groups
GSZ = C // G            # 64 channels / group
NPOS = GSZ * HW         # elements per group
NJT = HW // P           # 32 key tiles
EPS = 1e-6
SCALE = float(C) ** -0.5

F32 = mybir.dt.float32
BF16 = mybir.dt.bfloat16
AX = mybir.AxisListType
OP = mybir.AluOpType
AF = mybir.ActivationFunctionType


def _build(has_nw, has_nb, has_bq, has_bv, has_bp):
    nc = bacc.Bacc("TRN2", target_bir_lowering=False, debug=False,
                   num_devices=NCORES)

    xb_d = nc.dram_tensor("xb", [P, KC * HW], BF16, kind="ExternalInput").ap()
    xq_d = nc.dram_tensor("xq", [C, QB], F32, kind="ExternalInput").ap()
    xqb_d = nc.dram_tensor("xqb", [C, QB], BF16, kind="ExternalInput").ap()
    wt_d = nc.dram_tensor("wqkv", [3, C, C], F32, kind="ExternalInput").ap()
    wpt_d = nc.dram_tensor("wpt", [C, C], BF16, kind="ExternalInput").ap()
    ek_d = nc.dram_tensor("ek", [KC, P, G], F32, kind="ExternalInput").ap()
    ekb_d = nc.dram_tensor("ekb", [KC, P, G], BF16, kind="ExternalInput").ap()
    ones_d = nc.dram_tensor("ones32", [P, P], F32, kind="ExternalInput").ap()
    ekt_d = nc.dram_tensor("ekt", [KC, G, P], F32, kind="ExternalInput").ap()
    opt_d = {}
    for name, flag in (("nw", has_nw), ("nb", has_nb), ("bq", has_bq),
                       ("bv", has_bv), ("bp", has_bp)):
        if flag:
            opt_d[name] = nc.dram_tensor(
                name, [KC, P, 1], F32, kind="ExternalInput").ap()
    out_d = nc.dram_tensor("out", [C, QB], F32, kind="ExternalOutput").ap()

    with tile.TileContext(nc) as tc:
        _body(nc, tc, xb_d, xq_d, xqb_d, wt_d, wpt_d, ek_d, ekb_d, ekt_d,
              ones_d, opt_d, out_d, has_nw, has_nb, has_bq, has_bv, has_bp)

    nc.compile()
    return nc


def _body(nc, tc, xb_d, xq_d, xqb_d, wt_d, wpt_d, ek_d, ekb_d, ekt_d,
          ones_d, opt_d, out_d, has_nw, has_nb, has_bq, has_bv, has_bp):
    with (
        tc.tile_pool(name="kbuf", bufs=KC) as pk,
        tc.tile_pool(name="vt", bufs=1) as pvt,
        tc.tile_pool(name="qbuf", bufs=KC) as pq,
        tc.tile_pool(name="wp", bufs=1) as pwp,
        tc.tile_pool(name="xq", bufs=1) as pxq,
        tc.tile_pool(name="xqb", bufs=1) as pxqb,
        tc.tile_pool(name="small", bufs=4) as ps,
    ):
        # ---- persistent tiles (packed; few big DMAs) -------------------
        k_bf = [pk.tile([P, HW], BF16, tag="kbuf", name=f"kbf{k}") for k in range(KC)]
        vt_bf = pvt.tile([P, NJT * C], BF16, name="vtbf")
        q_bf = [pq.tile([P, QB], BF16, tag="qbuf", name=f"qbf{k}") for k in range(KC)]

        wpt_b = pwp.tile([P, KC, C], BF16, name="wptb")
        nc.gpsimd.dma_start(out=wpt_b[:],
                            in_=wpt_d.rearrange("(k p) n -> p k n", p=P))
        wpt_t = [wpt_b[:, k, :] for k in range(KC)]
        xq_b = pxq.tile([P, KC, QB], F32, name="xqb32")
        nc.gpsimd.dma_start(out=xq_b[:],
                            in_=xq_d.rearrange("(k p) n -> p k n", p=P))
        xq_t = [xq_b[:, k, :] for k in range(KC)]
        xqb_b = pxqb.tile([P, KC, QB], BF16, name="xqbb")
        nc.gpsimd.dma_start(out=xqb_b[:],
                            in_=xqb_d.rearrange("(k p) n -> p k n", p=P))
        xqb_t = [xqb_b[:, k, :] for k in range(KC)]
        ek_b = ps.tile([P, KC, G], F32, tag="ek", name="ekb")
        nc.gpsimd.dma_start(out=ek_b[:], in_=ek_d.rearrange("k p g -> p k g"))
        ek_t = [ek_b[:, k, :] for k in range(KC)]
        ekb_b = ps.tile([P, KC, G], BF16, tag="ekbf", name="ekbb")
        nc.gpsimd.dma_start(out=ekb_b[:], in_=ekb_d.rearrange("k p g -> p k g"))
        ekb_t = [ekb_b[:, k, :] for k in range(KC)]
        ones_t = ps.tile([P, P], F32, tag="ones", name="ones")
        nc.gpsimd.dma_start(out=ones_t[:], in_=ones_d[:])
        ekt_b = ps.tile([G, KC, P], F32, tag="ekt", name="ektb")
        nc.gpsimd.dma_start(out=ekt_b[:], in_=ekt_d.rearrange("k g p -> g k p"))
        ekt_t = [ekt_b[:, k, :] for k in range(KC)]
        opt_t = {}
        for name, ap in opt_d.items():
            ob = ps.tile([P, KC, 1], F32, tag=f"opt{name}", name=f"opt{name}b")
            nc.gpsimd.dma_start(out=ob[:], in_=ap.rearrange("k p o -> p k o"))
            opt_t[name] = [ob[:, k, :] for k in range(KC)]

        # per-channel scale (rstd*norm_w) and t/s (= -mean + norm_b/s)
        ch_t = [ps.tile([P, 2], F32, tag="ch", name=f"ch{k}") for k in range(KC)]
        scale_t = [ps.tile([P, 1], F32, tag="scale", name=f"scl{k}") for k in range(KC)]
        bos_t = [ps.tile([P, 1], BF16, tag="bos", name=f"bos{k}") for k in range(KC)]
        bqe_t = [ps.tile([P, 1], F32, tag="bqe", name=f"bqe{k}") for k in range(KC)]
        bve_t = [ps.tile([P, 1], BF16, tag="bve", name=f"bve{k}") for k in range(KC)]
        bpe_t = [ps.tile([P, 1], F32, tag="bpe", name=f"bpe{k}") for k in range(KC)]

        with (
            tc.tile_pool(name="xbuf", bufs=1) as px,
            tc.tile_pool(name="wf32", bufs=1) as pwf,
            tc.tile_pool(name="statps", bufs=1, space="PSUM") as pssm,
        ):
            x_b = px.tile([P, KC, HW], BF16, name="xbig")
            x_bf = [x_b[:, k, :] for k in range(KC)]
            xb_v = xb_d.rearrange("p (k n) -> p k n", k=KC)
            NQT = 4
            QTR = HW // NQT
            for k in range(KC):
                nc.sync.dma_start(out=x_b[:, k, :], in_=xb_v[:, k, :])
            wf_b = pwf.tile([P, 3, KC, C], F32, name="wfb")
            nc.sync.dma_start(
                out=wf_b[:], in_=wt_d.rearrange("w (k p) n -> p w k n", p=P))
            wf_t = {w: [wf_b[:, wi, k, :] for k in range(KC)]
                    for wi, w in enumerate("qkv")}

            # ---- group stats (pipelined with the DMA) ------------------
            # s1 per group via indicator matmuls on PE (accumulating over
            # chunks AND position tiles into one [G, 512] psum), s2 via
            # x*x sum-reductions split across DVE and ACT.
            eps_t = ps.tile([G, 1], F32, tag="eps", name="eps")
            nc.gpsimd.memset(eps_t[:], float(EPS))
            warm = ps.tile([G, 1], F32, tag="warm", name="warm")
            nc.scalar.activation(out=warm[:], in_=eps_t[:], func=AF.Sqrt,
                                 bias=eps_t[:])
            nc.scalar.activation(out=warm[:], in_=eps_t[:], func=AF.Exp,
                                 scale=SCALE)

            s1ps = pssm.tile([G, 512], F32, tag="gps", name="s1ps")
            s2g = pssm.tile([G, 1], F32, tag="s2g", name="s2g")
            sqq_t = [ps.tile([P, NQT], F32, tag="sqq", name=f"sqq{k}")
                     for k in range(KC)]
            NT = HW // 512
            idx = 0
            with tc.tile_pool(name="scratch", bufs=3) as psc:
                for k in range(KC):
                    for t in range(NT):
                        nc.tensor.matmul(
                            s1ps[:], lhsT=ekb_t[k][:],
                            rhs=x_bf[k][:, 512 * t:512 * (t + 1)],
                            start=(idx == 0), stop=(idx == KC * NT - 1))
                        idx += 1
                    for qt in range(NQT):
                        sl = slice(QTR * qt, QTR * (qt + 1))
                        scr = psc.tile([P, QTR], BF16, tag="scr",
                                       name=f"scr{k}{qt}")
                        if (k * NQT + qt) % 16 < 6:
                            nc.vector.tensor_tensor(
                                out=scr[:], in0=x_bf[k][:, sl],
                                in1=x_bf[k][:, sl], op=OP.mult)
                            nc.vector.tensor_reduce(
                                out=sqq_t[k][:, qt:qt + 1], in_=scr[:],
                                axis=AX.X, op=OP.add)
                        else:
                            nc.scalar.activation(
                                out=scr[:], in_=x_bf[k][:, sl],
                                func=AF.Square,
                                accum_out=sqq_t[k][:, qt:qt + 1])
                for k in range(KC):
                    s2ch = ps.tile([P, 1], F32, tag="s2ch", name=f"s2ch{k}")
                    nc.vector.tensor_reduce(
                        out=s2ch[:], in_=sqq_t[k][:], axis=AX.X, op=OP.add)
                    nc.tensor.matmul(s2g[:], lhsT=ek_t[k][:], rhs=s2ch[:],
                                     start=(k == 0), stop=(k == KC - 1))

            # mean/var/rstd per group
            gm = ps.tile([G, 2], F32, tag="gm", name="gm")
            nc.vector.tensor_reduce(
                out=gm[:, 0:1], in_=s1ps[:], axis=AX.X, op=OP.add)
            nc.vector.tensor_copy(out=gm[:, 1:2], in_=s2g[:])
            nc.vector.tensor_scalar_mul(gm[:], gm[:], 1.0 / NPOS)
            m2 = ps.tile([G, 1], F32, tag="m2", name="m2")
            nc.vector.tensor_tensor(
                out=m2[:], in0=gm[:, 0:1], in1=gm[:, 0:1], op=OP.mult)
            var = ps.tile([G, 1], F32, tag="var", name="var")
            nc.vector.tensor_tensor(
                out=var[:], in0=gm[:, 1:2], in1=m2[:], op=OP.subtract)
            std = ps.tile([G, 1], F32, tag="std", name="std")
            nc.scalar.activation(out=std[:], in_=var[:], func=AF.Sqrt,
                                 bias=eps_t[:])
            gb = ps.tile([G, 2], F32, tag="gb", name="gb")
            nc.vector.tensor_copy(out=gb[:, 0:1], in_=gm[:, 0:1])
            nc.vector.reciprocal(out=gb[:, 1:2], in_=std[:])

            # broadcast group stats back to channels
            for k in range(KC):
                bcp = pssm.tile([P, 2], F32, tag="bcp", name=f"bcp{k}")
                nc.tensor.matmul(bcp[:], lhsT=ekt_t[k][:], rhs=gb[:],
                                 start=True, stop=True)
                nc.vector.tensor_copy(out=ch_t[k][:], in_=bcp[:])
                if has_nw:
                    nc.vector.tensor_tensor(
                        out=scale_t[k][:], in0=ch_t[k][:, 1:2],
                        in1=opt_t["nw"][k][:], op=OP.mult)
                else:
                    nc.vector.tensor_copy(
                        out=scale_t[k][:], in_=ch_t[k][:, 1:2])
                # bos = t/s = -mean (+ norm_b / s)
                if has_nb:
                    rs = ps.tile([P, 1], F32, tag="rs", name=f"rs{k}")
                    nc.vector.reciprocal(out=rs[:], in_=scale_t[k][:])
                    nc.vector.tensor_tensor(
                        out=rs[:], in0=rs[:], in1=opt_t["nb"][k][:],
                        op=OP.mult)
                    nc.vector.scalar_tensor_tensor(
                        out=bos_t[k][:], in0=ch_t[k][:, 0:1], scalar=-1.0,
                        in1=rs[:], op0=OP.mult, op1=OP.add)
                else:
                    nc.vector.tensor_scalar_mul(
                        bos_t[k][:], ch_t[k][:, 0:1], -1.0)

            # ---- scaled weights + effective biases + convs -------------
            with (
                tc.tile_pool(name="wqkv", bufs=KC) as pw,
                tc.tile_pool(name="convps", bufs=4, space="PSUM") as pcv,
            ):
                ws = {}
                for wi, w in enumerate("qkv"):
                    ws[w] = [pw.tile([P, C], BF16, tag=f"w{w}", name=f"w{w}{k}")
                             for k in range(KC)]
                    for k in range(KC):
                        nc.scalar.activation(
                            out=ws[w][k][:], in_=wf_t[w][k][:], func=AF.Copy,
                            scale=scale_t[k][:])

                # K = wk_s.T @ x, laid out [cout, j] (no bias - cancels)
                for m in range(KC):
                    for t in range(HW // 512):
                        kp = pcv.tile([P, 512], F32, tag="cv", name=f"kp{m}{t}")
                        for k in range(KC):
                            nc.tensor.matmul(
                                kp[:],
                                lhsT=ws["k"][k][:, P * m:P * (m + 1)],
                                rhs=x_bf[k][:, 512 * t:512 * (t + 1)],
                                start=(k == 0), stop=(k == KC - 1))
                        nc.vector.tensor_copy(
                            out=k_bf[m][:, 512 * t:512 * (t + 1)], in_=kp[:])

                # VT = x.T @ wv_s, laid out [j, cout] in 32 j-tiles
                for jt in range(NJT):
                    vp = pcv.tile([P, 512], F32, tag="cv", name=f"vp{jt}")
                    for k in range(KC):
                        nc.tensor.matmul(
                            vp[:],
                            lhsT=x_bf[k][:, P * jt:P * (jt + 1)],
                            rhs=ws["v"][k][:],
                            start=(k == 0), stop=(k == KC - 1))
                    nc.vector.tensor_copy(
                        out=vt_bf[:, C * jt:C * (jt + 1)], in_=vp[:])

                # effective biases: beff_X[cout] = sum_cin wXs[cin,cout]*bos[cin]
                def beff(wtiles, dst, extra):
                    for m in range(KC):
                        bp_ps = pssm.tile([P, 1], F32, tag="beffps", name=f"bps{m}")
                        for k in range(KC):
                            nc.tensor.matmul(
                                bp_ps[:],
                                lhsT=wtiles[k][:, P * m:P * (m + 1)],
                                rhs=bos_t[k][:],
                                start=(k == 0), stop=(k == KC - 1))
                        if extra is not None:
                            nc.vector.tensor_tensor(
                                out=dst[m][:], in0=bp_ps[:],
                                in1=extra[m][:], op=OP.add)
                        else:
                            nc.vector.tensor_copy(out=dst[m][:], in_=bp_ps[:])

                beff(ws["q"], bqe_t, opt_t.get("bq"))
                beff(ws["v"], bve_t, opt_t.get("bv"))
                # fold v-bias through proj: bpe = wp @ bve (+ bp)
                for m in range(KC):
                    bp_ps = pssm.tile([P, 1], F32, tag="beffps", name=f"bpp{m}")
                    for k in range(KC):
                        nc.tensor.matmul(
                            bp_ps[:],
                            lhsT=wpt_t[k][:, P * m:P * (m + 1)],
                            rhs=bve_t[k][:],
                            start=(k == 0), stop=(k == KC - 1))
                    if has_bp:
                        nc.vector.tensor_tensor(
                            out=bpe_t[m][:], in0=bp_ps[:],
                            in1=opt_t["bp"][m][:], op=OP.add)
                    else:
                        nc.vector.tensor_copy(out=bpe_t[m][:], in_=bp_ps[:])

                # Q = wq_s.T @ xq (+bq_eff), laid out [cout, i]
                for m in range(KC):
                    for t in range(NIH):
                        qp = pcv.tile([P, 512], F32, tag="cv", name=f"qp{m}{t}")
                        for k in range(KC):
                            nc.tensor.matmul(
                                qp[:],
                                lhsT=ws["q"][k][:, P * m:P * (m + 1)],
                                rhs=xqb_t[k][:, 512 * t:512 * (t + 1)],
                                start=(k == 0), stop=(k == KC - 1))
                        nc.vector.tensor_scalar_add(
                            q_bf[m][:, 512 * t:512 * (t + 1)],
                            qp[:], bqe_t[m][:])


        # ---- attention ---------------------------------------------
        with (
            tc.tile_pool(name="at", bufs=6) as pa,
            tc.tile_pool(name="obuf", bufs=2 * KC) as po,
            tc.tile_pool(name="rb", bufs=2) as prb,
            tc.tile_pool(name="outb", bufs=2) as pob,
            tc.tile_pool(name="acc", bufs=2) as pacc,
            tc.tile_pool(name="sps", bufs=3, space="PSUM") as psps,
            tc.tile_pool(name="ops", bufs=4, space="PSUM") as pops,
            tc.tile_pool(name="csps", bufs=1, space="PSUM") as pcs,
        ):
            for ih in range(NIH):
                i_sl = slice(512 * ih, 512 * (ih + 1))
                o_ps = [pops.tile([P, 512], F32, tag="ops", name=f"ops{m}")
                        for m in range(KC)]
                acc = pacc.tile([P, 512], F32, tag="acc", name=f"acc{ih}")
                ats = [None] * NJT

                LAG = 4

                def tail(jt):
                    # O[c] += VT[jt].T @ A
                    for m in range(KC):
                        nc.tensor.matmul(
                            o_ps[m][:],
                            lhsT=vt_bf[:, C * jt + P * m:C * jt + P * (m + 1)],
                            rhs=ats[jt][:],
                            start=(jt == 0), stop=(jt == NJT - 1))

                for jt in range(NJT):
                    sp = psps.tile([P, 512], F32, tag="sp", name=f"sp{jt}")
                    for k in range(KC):
                        nc.tensor.matmul(
                            sp[:],
                            lhsT=k_bf[k][:, P * jt:P * (jt + 1)],
                            rhs=q_bf[k][:, i_sl],
                            start=(k == 0), stop=(k == KC - 1))
                    at = pa.tile([P, 512], BF16, tag="at", name=f"at{jt}")
                    nc.scalar.activation(out=at[:], in_=sp[:], func=AF.Exp,
                                         scale=SCALE)
                    ats[jt] = at
                    if jt == 0:
                        nc.vector.tensor_copy(out=acc[:], in_=at[:])
                    else:
                        nc.vector.tensor_tensor(
                            out=acc[:], in0=acc[:], in1=at[:], op=OP.add)
                    if jt >= LAG:
                        tail(jt - LAG)
                for jt in range(NJT - LAG, NJT):
                    tail(jt)

                # normalize rows, then proj + residual
                cs_ps = pcs.tile([P, 512], F32, tag="cs", name=f"cs{ih}")
                nc.tensor.matmul(cs_ps[:], lhsT=ones_t[:], rhs=acc[:],
                                 start=True, stop=True)
                rb = prb.tile([P, 512], F32, tag="rb", name="rb")
                nc.vector.reciprocal_approx_fast(out=rb[:], in_=cs_ps[:])
                o_t = [po.tile([P, 512], BF16, tag="ob", name=f"ot{m}")
                       for m in range(KC)]
                for m in range(KC):
                    nc.vector.tensor_tensor(
                        out=o_t[m][:], in0=o_ps[m][:], in1=rb[:], op=OP.mult)
                ob = pob.tile([P, KC, 512], F32, tag="outb", name=f"outt{ih}")
                for m in range(KC):
                    pp = pops.tile([P, 512], F32, tag="ops", name=f"pp{m}")
                    for k in range(KC):
                        nc.tensor.matmul(
                            pp[:],
                            lhsT=wpt_t[k][:, P * m:P * (m + 1)],
                            rhs=o_t[k][:],
                            start=(k == 0), stop=(k == KC - 1))
                    nc.vector.scalar_tensor_tensor(
                        out=ob[:, m, :], in0=pp[:], scalar=bpe_t[m][:],
                        in1=xq_t[m][:, i_sl], op0=OP.add, op1=OP.add)
                nc.sync.dma_start(
                    out=out_d.rearrange("(k p) n -> p k n", p=P)[:, :, i_sl],
                    in_=ob[:])


_NC_CACHE = {}


def _get_nc(flags):
    if flags not in _NC_CACHE:
        _NC_CACHE[flags] = _build(*flags)
    return _NC_CACHE[flags]


def _host_consts():
    ek = np.zeros((KC, P, G), np.float32)
    for k in range(KC):
        for p in range(P):
            ek[k, p, (p + P * k) // GSZ] = 1.0
    ekt = np.ascontiguousarray(ek.transpose(0, 2, 1))
    return ek, ekt


def prepare(inputs):
    x = np.ascontiguousarray(np.asarray(inputs["x"], np.float32))
    norm_w = np.asarray(inputs["norm_w"], np.float32)
    norm_b = np.asarray(inputs["norm_b"], np.float32)
    wts = {w: np.ascontiguousarray(
        np.asarray(inputs["w" + w], np.float32).T) for w in "qkvp"}
    bs = {w: np.asarray(inputs["b" + w], np.float32) for w in "qkvp"}
    wpt_bf = wts["p"].astype(ml_dtypes.bfloat16)
    wqkv = np.ascontiguousarray(np.stack([wts["q"], wts["k"], wts["v"]]))

    flags = (bool(np.any(norm_w != 1.0)), bool(np.any(norm_b != 0.0)),
             bool(np.any(bs["q"] != 0.0)), bool(np.any(bs["v"] != 0.0)),
             bool(np.any(bs["p"] != 0.0)))
    ek, ekt = _host_consts()
    in_maps = []
    for core in range(NCORES):
        b, qb = divmod(core, NCORES // B)
        xb = np.ascontiguousarray(x[b].reshape(C, HW))
        xq = np.ascontiguousarray(xb[:, qb * QB:(qb + 1) * QB])
        xbp = np.ascontiguousarray(
            xb.reshape(KC, P, HW).transpose(1, 0, 2).reshape(P, KC * HW))
        m = {
            "xb": xbp.astype(ml_dtypes.bfloat16),
            "xq": xq,
            "xqb": xq.astype(ml_dtypes.bfloat16),
            "wqkv": wqkv, "wpt": wpt_bf,
            "ek": ek, "ekb": ek.astype(ml_dtypes.bfloat16), "ekt": ekt,
            "ones32": np.ones((P, P), np.float32),
        }
        for name, flag, arr in (("nw", flags[0], norm_w), ("nb", flags[1], norm_b),
                                ("bq", flags[2], bs["q"]), ("bv", flags[3], bs["v"]),
                                ("bp", flags[4], bs["p"])):
            if flag:
                m[name] = np.ascontiguousarray(arr.reshape(KC, P, 1))
        in_maps.append(m)
    return flags, in_maps


def assemble(results):
    out = np.empty((B, C, HW), np.float32)
    for core in range(NCORES):
        b, qb = divmod(core, NCORES // B)
        out[b][:, qb * QB:(qb + 1) * QB] = results[core]["out"]
    return out.reshape(B, C, H, W)


def run(inputs, **spmd_kwargs):
    flags, in_maps = prepare(inputs)
    nc = _get_nc(flags)
    res = bass_utils.run_bass_kernel_spmd(nc, in_maps, list(range(NCORES)),
                                          **spmd_kwargs)
    return assemble(res.results), res


def kernel(**inputs):
    out, _ = run(inputs)
    return out


# revision 24
# speedup vs baseline: 1.0152x; 1.0152x over previous
"""Trainium2 Bass kernel: VAE-style AttnBlock.

  y = x + proj( attention( q(gn(x)), k(gn(x)), v(gn(x)) ) )

  x: [2, 512, 64, 64] f32, gn = GroupNorm(8 groups, eps=1e-6),
  q/k/v/proj = 1x1 convs (512x512), attention over the 4096 spatial
  positions with softmax along the key axis, scale = 512**-0.5.

Sharding: 8 cores = (batch b, query-block qb); each core computes the
softmax rows for its 1024 query positions of batch b against the full
K/V of that batch (K/V conv is recomputed per core - cheaper than a
cross-core exchange at this size). Conv weights replicated.

Device-side structure: GroupNorm is folded into the conv weights.
  xn[c,:] = x[c,:]*s_c + t_c   with s_c = rstd_g*norm_w_c,
                                    t_c = norm_b_c - mean_g*s_c
  conv(xn) = (W*s) @ x + (W @ t + b)
After computing group stats on device, the transposed conv weights are
scaled by s per input-channel (cast to bf16), and effective biases are
computed with tiny matmuls (rhs = t/s, against the scaled weights).
The k-bias is skipped: softmax_j((Q0+bq).(K0+bk)) = softmax_j((Q0+bq).K0)
since the bk term only adds a per-row constant. The v-bias (sum of the
softmax weights times a constant = the constant) is folded through the
proj conv into the output bias.

Softmax runs without max-subtraction: logits here are ~N(0,1) after the
1/sqrt(C) scale, so exp stays comfortably finite in fp32.

Matmul dtype is bf16 with fp32 PSUM accumulation throughout (incl. Q@K
and A@V); the softmax normalizer, proj epilogue and residual are fp32.
"""

import numpy as np
import ml_dtypes

import concourse.bacc as bacc
import concourse.tile as tile
from concourse import mybir
from concourse import bass_utils

B, C, H, W = 2, 512, 64, 64
HW = H * W              # 4096 spatial positions
P = 128                 # partitions
KC = C // P             # 4 channel chunks
NCORES = 8
QB = B * HW // NCORES   # 1024 query positions per core
NIH = 2                 # query halves of 512
G = 8                   # groups
GSZ = C // G            # 64 channels / group
NPOS = GSZ * HW         # elements per group
NJT = HW // P           # 32 key tiles
EPS = 1e-6
SCALE = float(C) ** -0.5

F32 = mybir.dt.float32
BF16 = mybir.dt.bfloat16
AX = mybir.AxisListType
OP = mybir.AluOpType
AF = mybir.ActivationFunctionType


def _build(has_nw, has_nb, has_bq, has_bv, has_bp):
    nc = bacc.Bacc("TRN2", target_bir_lowering=False, debug=False,
                   num_devices=NCORES)

    xb_d = nc.dram_tensor("xb", [C, HW], BF16, kind="ExternalInput").ap()
    xq_d = nc.dram_tensor("xq", [C, QB], F32, kind="ExternalInput").ap()
    xqb_d = nc.dram_tensor("xqb", [C, QB], BF16, kind="ExternalInput").ap()
    wt_d = nc.dram_tensor("wqkv", [3, C, C], F32, kind="ExternalInput").ap()
    wpt_d = nc.dram_tensor("wpt", [C, C], BF16, kind="ExternalInput").ap()
    ek_d = nc.dram_tensor("ek", [KC, P, G], F32, kind="ExternalInput").ap()
    ekb_d = nc.dram_tensor("ekb", [KC, P, G], BF16, kind="ExternalInput").ap()
    ones_d = nc.dram_tensor("ones32", [P, P], F32, kind="ExternalInput").ap()
    ekt_d = nc.dram_tensor("ekt", [KC, G, P], F32, kind="ExternalInput").ap()
    opt_d = {}
    for name, flag in (("nw", has_nw), ("nb", has_nb), ("bq", has_bq),
                       ("bv", has_bv), ("bp", has_bp)):
        if flag:
            opt_d[name] = nc.dram_tensor(
                name, [KC, P, 1], F32, kind="ExternalInput").ap()
    out_d = nc.dram_tensor("out", [C, QB], F32, kind="ExternalOutput").ap()

    with tile.TileContext(nc) as tc:
        _body(nc, tc, xb_d, xq_d, xqb_d, wt_d, wpt_d, ek_d, ekb_d, ekt_d,
              ones_d, opt_d, out_d, has_nw, has_nb, has_bq, has_bv, has_bp)

    nc.compile()
    return nc


def _body(nc, tc, xb_d, xq_d, xqb_d, wt_d, wpt_d, ek_d, ekb_d, ekt_d,
          ones_d, opt_d, out_d, has_nw, has_nb, has_bq, has_bv, has_bp):
    with (
        tc.tile_pool(name="kbuf", bufs=KC) as pk,
        tc.tile_pool(name="vt", bufs=1) as pvt,
        tc.tile_pool(name="qbuf", bufs=KC) as pq,
        tc.tile_pool(name="wp", bufs=1) as pwp,
        tc.tile_pool(name="xq", bufs=1) as pxq,
        tc.tile_pool(name="xqb", bufs=1) as pxqb,
        tc.tile_pool(name="small", bufs=4) as ps,
    ):
        # ---- persistent tiles (packed; few big DMAs) -------------------
        k_bf = [pk.tile([P, HW], BF16, tag="kbuf", name=f"kbf{k}") for k in range(KC)]
        vt_bf = pvt.tile([P, NJT * C], BF16, name="vtbf")
        q_bf = [pq.tile([P, QB], BF16, tag="qbuf", name=f"qbf{k}") for k in range(KC)]

        wpt_b = pwp.tile([P, KC, C], BF16, name="wptb")
        nc.gpsimd.dma_start(out=wpt_b[:],
                            in_=wpt_d.rearrange("(k p) n -> p k n", p=P))
        wpt_t = [wpt_b[:, k, :] for k in range(KC)]
        xq_b = pxq.tile([P, KC, QB], F32, name="xqb32")
        nc.gpsimd.dma_start(out=xq_b[:],
                            in_=xq_d.rearrange("(k p) n -> p k n", p=P))
        xq_t = [xq_b[:, k, :] for k in range(KC)]
        xqb_b = pxqb.tile([P, KC, QB], BF16, name="xqbb")
        nc.gpsimd.dma_start(out=xqb_b[:],
                            in_=xqb_d.rearrange("(k p) n -> p k n", p=P))
        xqb_t = [xqb_b[:, k, :] for k in range(KC)]
        ek_b = ps.tile([P, KC, G], F32, tag="ek", name="ekb")
        nc.gpsimd.dma_start(out=ek_b[:], in_=ek_d.rearrange("k p g -> p k g"))
        ek_t = [ek_b[:, k, :] for k in range(KC)]
        ekb_b = ps.tile([P, KC, G], BF16, tag="ekbf", name="ekbb")
        nc.gpsimd.dma_start(out=ekb_b[:], in_=ekb_d.rearrange("k p g -> p k g"))
        ekb_t = [ekb_b[:, k, :] for k in range(KC)]
        ones_t = ps.tile([P, P], F32, tag="ones", name="ones")
        nc.gpsimd.dma_start(out=ones_t[:], in_=ones_d[:])
        ekt_b = ps.tile([G, KC, P], F32, tag="ekt", name="ektb")
        nc.gpsimd.dma_start(out=ekt_b[:], in_=ekt_d.rearrange("k g p -> g k p"))
        ekt_t = [ekt_b[:, k, :] for k in range(KC)]
        opt_t = {}
        for name, ap in opt_d.items():
            ob = ps.tile([P, KC, 1], F32, tag=f"opt{name}", name=f"opt{name}b")
            nc.gpsimd.dma_start(out=ob[:], in_=ap.rearrange("k p o -> p k o"))
            opt_t[name] = [ob[:, k, :] for k in range(KC)]

        # per-channel scale (rstd*norm_w) and t/s (= -mean + norm_b/s)
        ch_t = [ps.tile([P, 2], F32, tag="ch", name=f"ch{k}") for k in range(KC)]
        scale_t = [ps.tile([P, 1], F32, tag="scale", name=f"scl{k}") for k in range(KC)]
        bos_t = [ps.tile([P, 1], BF16, tag="bos", name=f"bos{k}") for k in range(KC)]
        bqe_t = [ps.tile([P, 1], F32, tag="bqe", name=f"bqe{k}") for k in range(KC)]
        bve_t = [ps.tile([P, 1], BF16, tag="bve", name=f"bve{k}") for k in range(KC)]
        bpe_t = [ps.tile([P, 1], F32, tag="bpe", name=f"bpe{k}") for k in range(KC)]

        with (
            tc.tile_pool(name="xbuf", bufs=1) as px,
            tc.tile_pool(name="wf32", bufs=1) as pwf,
            tc.tile_pool(name="statps", bufs=1, space="PSUM") as pssm,
        ):
            x_b = px.tile([P, KC, HW], BF16, name="xbig")
            x_bf = [x_b[:, k, :] for k in range(KC)]
            xb_v = xb_d.rearrange("(k p) n -> p k n", p=P)
            NQT = 4
            QTR = HW // NQT
            for qt in range(NQT):
                sl = slice(QTR * qt, QTR * (qt + 1))
                nc.sync.dma_start(out=x_b[:, :, sl], in_=xb_v[:, :, sl])
            wf_b = pwf.tile([P, 3, KC, C], F32, name="wfb")
            nc.sync.dma_start(
                out=wf_b[:], in_=wt_d.rearrange("w (k p) n -> p w k n", p=P))
            wf_t = {w: [wf_b[:, wi, k, :] for k in range(KC)]
                    for wi, w in enumerate("qkv")}

            # ---- group stats (pipelined with the DMA) ------------------
            # s1 per group via indicator matmuls on PE (accumulating over
            # chunks AND position tiles into one [G, 512] psum), s2 via
            # x*x sum-reductions split across DVE and ACT.
            eps_t = ps.tile([G, 1], F32, tag="eps", name="eps")
            nc.gpsimd.memset(eps_t[:], float(EPS))
            warm = ps.tile([G, 1], F32, tag="warm", name="warm")
            nc.scalar.activation(out=warm[:], in_=eps_t[:], func=AF.Sqrt,
                                 bias=eps_t[:])
            nc.scalar.activation(out=warm[:], in_=eps_t[:], func=AF.Exp,
                                 scale=SCALE)

            s1ps = pssm.tile([G, 512], F32, tag="gps", name="s1ps")
            s2g = pssm.tile([G, 1], F32, tag="s2g", name="s2g")
            sqq_t = [ps.tile([P, NQT], F32, tag="sqq", name=f"sqq{k}")
                     for k in range(KC)]
            NT = HW // 512
            TPQ = NT // NQT
            idx = 0
            with tc.tile_pool(name="scratch", bufs=3) as psc:
                for qt in range(NQT):
                    for tt in range(TPQ):
                        t = qt * TPQ + tt
                        for k in range(KC):
                            nc.tensor.matmul(
                                s1ps[:], lhsT=ekb_t[k][:],
                                rhs=x_bf[k][:, 512 * t:512 * (t + 1)],
                                start=(idx == 0), stop=(idx == KC * NT - 1))
                            idx += 1
                    for k in range(KC):
                        sl = slice(QTR * qt, QTR * (qt + 1))
                        scr = psc.tile([P, QTR], BF16, tag="scr",
                                       name=f"scr{k}{qt}")
                        if (qt * KC + k) % 16 < 7:
                            nc.vector.tensor_tensor(
                                out=scr[:], in0=x_bf[k][:, sl],
                                in1=x_bf[k][:, sl], op=OP.mult)
                            nc.vector.tensor_reduce(
                                out=sqq_t[k][:, qt:qt + 1], in_=scr[:],
                                axis=AX.X, op=OP.add)
                        else:
                            nc.scalar.activation(
                                out=scr[:], in_=x_bf[k][:, sl],
                                func=AF.Square,
                                accum_out=sqq_t[k][:, qt:qt + 1])
                for k in range(KC):
                    s2ch = ps.tile([P, 1], F32, tag="s2ch", name=f"s2ch{k}")
                    nc.vector.tensor_reduce(
                        out=s2ch[:], in_=sqq_t[k][:], axis=AX.X, op=OP.add)
                    nc.tensor.matmul(s2g[:], lhsT=ek_t[k][:], rhs=s2ch[:],
                                     start=(k == 0), stop=(k == KC - 1))

            # mean/var/rstd per group
            gm = ps.tile([G, 2], F32, tag="gm", name="gm")
            nc.vector.tensor_reduce(
                out=gm[:, 0:1], in_=s1ps[:], axis=AX.X, op=OP.add)
            nc.vector.tensor_copy(out=gm[:, 1:2], in_=s2g[:])
            nc.vector.tensor_scalar_mul(gm[:], gm[:], 1.0 / NPOS)
            m2 = ps.tile([G, 1], F32, tag="m2", name="m2")
            nc.vector.tensor_tensor(
                out=m2[:], in0=gm[:, 0:1], in1=gm[:, 0:1], op=OP.mult)
            var = ps.tile([G, 1], F32, tag="var", name="var")
            nc.vector.tensor_tensor(
                out=var[:], in0=gm[:, 1:2], in1=m2[:], op=OP.subtract)
            std = ps.tile([G, 1], F32, tag="std", name="std")
            nc.scalar.activation(out=std[:], in_=var[:], func=AF.Sqrt,
                                 bias=eps_t[:])
            gb = ps.tile([G, 2], F32, tag="gb", name="gb")
            nc.vector.tensor_copy(out=gb[:, 0:1], in_=gm[:, 0:1])
            nc.vector.reciprocal(out=gb[:, 1:2], in_=std[:])

            # broadcast group stats back to channels
            for k in range(KC):
                bcp = pssm.tile([P, 2], F32, tag="bcp", name=f"bcp{k}")
                nc.tensor.matmul(bcp[:], lhsT=ekt_t[k][:], rhs=gb[:],
                                 start=True, stop=True)
                nc.vector.tensor_copy(out=ch_t[k][:], in_=bcp[:])
                if has_nw:
                    nc.vector.tensor_tensor(
                        out=scale_t[k][:], in0=ch_t[k][:, 1:2],
                        in1=opt_t["nw"][k][:], op=OP.mult)
                else:
                    nc.vector.tensor_copy(
                        out=scale_t[k][:], in_=ch_t[k][:, 1:2])
                # bos = t/s = -mean (+ norm_b / s)
                if has_nb:
                    rs = ps.tile([P, 1], F32, tag="rs", name=f"rs{k}")
                    nc.vector.reciprocal(out=rs[:], in_=scale_t[k][:])
                    nc.vector.tensor_tensor(
                        out=rs[:], in0=rs[:], in1=opt_t["nb"][k][:],
                        op=OP.mult)
                    nc.vector.scalar_tensor_tensor(
                        out=bos_t[k][:], in0=ch_t[k][:, 0:1], scalar=-1.0,
                        in1=rs[:], op0=OP.mult, op1=OP.add)
                else:
                    nc.vector.tensor_scalar_mul(
                        bos_t[k][:], ch_t[k][:, 0:1], -1.0)

            # ---- scaled weights + effective biases + convs -------------
            with (
                tc.tile_pool(name="wqkv", bufs=KC) as pw,
                tc.tile_pool(name="convps", bufs=4, space="PSUM") as pcv,
            ):
                ws = {}
                for wi, w in enumerate("qkv"):
                    ws[w] = [pw.tile([P, C], BF16, tag=f"w{w}", name=f"w{w}{k}")
                             for k in range(KC)]
                    for k in range(KC):
                        nc.scalar.activation(
                            out=ws[w][k][:], in_=wf_t[w][k][:], func=AF.Copy,
                            scale=scale_t[k][:])

                # K = wk_s.T @ x, laid out [cout, j] (no bias - cancels)
                for m in range(KC):
                    for t in range(HW // 512):
                        kp = pcv.tile([P, 512], F32, tag="cv", name=f"kp{m}{t}")
                        for k in range(KC):
                            nc.tensor.matmul(
                                kp[:],
                                lhsT=ws["k"][k][:, P * m:P * (m + 1)],
                                rhs=x_bf[k][:, 512 * t:512 * (t + 1)],
                                start=(k == 0), stop=(k == KC - 1))
                        nc.vector.tensor_copy(
                            out=k_bf[m][:, 512 * t:512 * (t + 1)], in_=kp[:])

                # VT = x.T @ wv_s, laid out [j, cout] in 32 j-tiles
                for jt in range(NJT):
                    vp = pcv.tile([P, 512], F32, tag="cv", name=f"vp{jt}")
                    for k in range(KC):
                        nc.tensor.matmul(
                            vp[:],
                            lhsT=x_bf[k][:, P * jt:P * (jt + 1)],
                            rhs=ws["v"][k][:],
                            start=(k == 0), stop=(k == KC - 1))
                    nc.vector.tensor_copy(
                        out=vt_bf[:, C * jt:C * (jt + 1)], in_=vp[:])

                # effective biases: beff_X[cout] = sum_cin wXs[cin,cout]*bos[cin]
                def beff(wtiles, dst, extra):
                    for m in range(KC):
                        bp_ps = pssm.tile([P, 1], F32, tag="beffps", name=f"bps{m}")
                        for k in range(KC):
                            nc.tensor.matmul(
                                bp_ps[:],
                                lhsT=wtiles[k][:, P * m:P * (m + 1)],
                                rhs=bos_t[k][:],
                                start=(k == 0), stop=(k == KC - 1))
                        if extra is not None:
                            nc.vector.tensor_tensor(
                                out=dst[m][:], in0=bp_ps[:],
                                in1=extra[m][:], op=OP.add)
                        else:
                            nc.vector.tensor_copy(out=dst[m][:], in_=bp_ps[:])

                beff(ws["q"], bqe_t, opt_t.get("bq"))
                beff(ws["v"], bve_t, opt_t.get("bv"))
                # fold v-bias through proj: bpe = wp @ bve (+ bp)
                for m in range(KC):
                    bp_ps = pssm.tile([P, 1], F32, tag="beffps", name=f"bpp{m}")
                    for k in range(KC):
                        nc.tensor.matmul(
                            bp_ps[:],
                            lhsT=wpt_t[k][:, P * m:P * (m + 1)],
                            rhs=bve_t[k][:],
                            start=(k == 0), stop=(k == KC - 1))
                    if has_bp:
                        nc.vector.tensor_tensor(
                            out=bpe_t[m][:], in0=bp_ps[:],
                            in1=opt_t["bp"][m][:], op=OP.add)
                    else:
                        nc.vector.tensor_copy(out=bpe_t[m][:], in_=bp_ps[:])

                # Q = wq_s.T @ xq (+bq_eff), laid out [cout, i]
                for m in range(KC):
                    for t in range(NIH):
                        qp = pcv.tile([P, 512], F32, tag="cv", name=f"qp{m}{t}")
                        for k in range(KC):
                            nc.tensor.matmul(
                                qp[:],
                                lhsT=ws["q"][k][:, P * m:P * (m + 1)],
                                rhs=xqb_t[k][:, 512 * t:512 * (t + 1)],
                                start=(k == 0), stop=(k == KC - 1))
                        nc.vector.tensor_scalar_add(
                            q_bf[m][:, 512 * t:512 * (t + 1)],
                            qp[:], bqe_t[m][:])


        # ---- attention ---------------------------------------------
        with (
            tc.tile_pool(name="at", bufs=6) as pa,
            tc.tile_pool(name="obuf", bufs=2 * KC) as po,
            tc.tile_pool(name="rb", bufs=2) as prb,
            tc.tile_pool(name="outb", bufs=2) as pob,
            tc.tile_pool(name="acc", bufs=2) as pacc,
            tc.tile_pool(name="sps", bufs=3, space="PSUM") as psps,
            tc.tile_pool(name="ops", bufs=4, space="PSUM") as pops,
            tc.tile_pool(name="csps", bufs=1, space="PSUM") as pcs,
        ):
            for ih in range(NIH):
                i_sl = slice(512 * ih, 512 * (ih + 1))
                o_ps = [pops.tile([P, 512], F32, tag="ops", name=f"ops{m}")
                        for m in range(KC)]
                acc = pacc.tile([P, 512], F32, tag="acc", name=f"acc{ih}")
                ats = [None] * NJT

                LAG = 4

                def tail(jt):
                    # O[c] += VT[jt].T @ A
                    for m in range(KC):
                        nc.tensor.matmul(
                            o_ps[m][:],
                            lhsT=vt_bf[:, C * jt + P * m:C * jt + P * (m + 1)],
                            rhs=ats[jt][:],
                            start=(jt == 0), stop=(jt == NJT - 1))

                for jt in range(NJT):
                    sp = psps.tile([P, 512], F32, tag="sp", name=f"sp{jt}")
                    for k in range(KC):
                        nc.tensor.matmul(
                            sp[:],
                            lhsT=k_bf[k][:, P * jt:P * (jt + 1)],
                            rhs=q_bf[k][:, i_sl],
                            start=(k == 0), stop=(k == KC - 1))
                    at = pa.tile([P, 512], BF16, tag="at", name=f"at{jt}")
                    nc.scalar.activation(out=at[:], in_=sp[:], func=AF.Exp,
                                         scale=SCALE)
                    ats[jt] = at
                    if jt == 0:
                        nc.vector.tensor_copy(out=acc[:], in_=at[:])
                    else:
                        nc.vector.tensor_tensor(
                            out=acc[:], in0=acc[:], in1=at[:], op=OP.add)
                    if jt >= LAG:
                        tail(jt - LAG)
                for jt in range(NJT - LAG, NJT):
                    tail(jt)

                # normalize rows, then proj + residual
                cs_ps = pcs.tile([P, 512], F32, tag="cs", name=f"cs{ih}")
                nc.tensor.matmul(cs_ps[:], lhsT=ones_t[:], rhs=acc[:],
                                 start=True, stop=True)
                rb = prb.tile([P, 512], F32, tag="rb", name="rb")
                nc.vector.reciprocal_approx_fast(out=rb[:], in_=cs_ps[:])
                o_t = [po.tile([P, 512], BF16, tag="ob", name=f"ot{m}")
                       for m in range(KC)]
                for m in range(KC):
                    nc.vector.tensor_tensor(
                        out=o_t[m][:], in0=o_ps[m][:], in1=rb[:], op=OP.mult)
                ob = pob.tile([P, KC, 512], F32, tag="outb", name=f"outt{ih}")
                for m in range(KC):
                    pp = pops.tile([P, 512], F32, tag="ops", name=f"pp{m}")
                    for k in range(KC):
                        nc.tensor.matmul(
                            pp[:],
                            lhsT=wpt_t[k][:, P * m:P * (m + 1)],
                            rhs=o_t[k][:],
                            start=(k == 0), stop=(k == KC - 1))
                    nc.vector.scalar_tensor_tensor(
                        out=ob[:, m, :], in0=pp[:], scalar=bpe_t[m][:],
                        in1=xq_t[m][:, i_sl], op0=OP.add, op1=OP.add)
                nc.sync.dma_start(
                    out=out_d.rearrange("(k p) n -> p k n", p=P)[:, :, i_sl],
                    in_=ob[:])


_NC_CACHE = {}


def _get_nc(flags):
    if flags not in _NC_CACHE:
        _NC_CACHE[flags] = _build(*flags)
    return _NC_CACHE[flags]


def _host_consts():
    ek = np.zeros((KC, P, G), np.float32)
    for k in range(KC):
        for p in range(P):
            ek[k, p, (p + P * k) // GSZ] = 1.0
    ekt = np.ascontiguousarray(ek.transpose(0, 2, 1))
    return ek, ekt


def prepare(inputs):
    x = np.ascontiguousarray(np.asarray(inputs["x"], np.float32))
    norm_w = np.asarray(inputs["norm_w"], np.float32)
    norm_b = np.asarray(inputs["norm_b"], np.float32)
    wts = {w: np.ascontiguousarray(
        np.asarray(inputs["w" + w], np.float32).T) for w in "qkvp"}
    bs = {w: np.asarray(inputs["b" + w], np.float32) for w in "qkvp"}
    wpt_bf = wts["p"].astype(ml_dtypes.bfloat16)
    wqkv = np.ascontiguousarray(np.stack([wts["q"], wts["k"], wts["v"]]))

    flags = (bool(np.any(norm_w != 1.0)), bool(np.any(norm_b != 0.0)),
             bool(np.any(bs["q"] != 0.0)), bool(np.any(bs["v"] != 0.0)),
             bool(np.any(bs["p"] != 0.0)))
    ek, ekt = _host_consts()
    in_maps = []
    for core in range(NCORES):
        b, qb = divmod(core, NCORES // B)
        xb = np.ascontiguousarray(x[b].reshape(C, HW))
        xq = np.ascontiguousarray(xb[:, qb * QB:(qb + 1) * QB])
        m = {
            "xb": xb.astype(ml_dtypes.bfloat16),
            "xq": xq,
            "xqb": xq.astype(ml_dtypes.bfloat16),
            "wqkv": wqkv, "wpt": wpt_bf,
            "ek": ek, "ekb": ek.astype(ml_dtypes.bfloat16), "ekt": ekt,
            "ones32": np.ones((P, P), np.float32),
        }
        for name, flag, arr in (("nw", flags[0], norm_w), ("nb", flags[1], norm_b),
                                ("bq", flags[2], bs["q"]), ("bv", flags[3], bs["v"]),
                                ("bp", flags[4], bs["p"])):
            if flag:
                m[name] = np.ascontiguousarray(arr.reshape(KC, P, 1))
        in_maps.append(m)
    return flags, in_maps


def assemble(results):
    out = np.empty((B, C, HW), np.float32)
    for core in range(NCORES):
        b, qb = divmod(core, NCORES // B)
        out[b][:, qb * QB:(qb + 1) * QB] = results[core]["out"]
    return out.reshape(B, C, H, W)


def run(inputs, **spmd_kwargs):
    flags, in_maps = prepare(inputs)
    nc = _get_nc(flags)
    res = bass_utils.run_bass_kernel_spmd(nc, in_maps, list(range(NCORES)),
                                          **spmd_kwargs)
    return assemble(res.results), res


def kernel(**inputs):
    out, _ = run(inputs)
    return out


# revision 25
# speedup vs baseline: 1.0388x; 1.0232x over previous
"""Trainium2 Bass kernel: VAE-style AttnBlock.

  y = x + proj( attention( q(gn(x)), k(gn(x)), v(gn(x)) ) )

  x: [2, 512, 64, 64] f32, gn = GroupNorm(8 groups, eps=1e-6),
  q/k/v/proj = 1x1 convs (512x512), attention over the 4096 spatial
  positions with softmax along the key axis, scale = 512**-0.5.

Sharding: 8 cores = (batch b, query-block qb); each core computes the
softmax rows for its 1024 query positions of batch b against the full
K/V of that batch (K/V conv is recomputed per core - cheaper than a
cross-core exchange at this size). Conv weights replicated.

Device-side structure: GroupNorm is folded into the conv weights.
  xn[c,:] = x[c,:]*s_c + t_c   with s_c = rstd_g*norm_w_c,
                                    t_c = norm_b_c - mean_g*s_c
  conv(xn) = (W*s) @ x + (W @ t + b)
After computing group stats on device, the transposed conv weights are
scaled by s per input-channel (cast to bf16), and effective biases are
computed with tiny matmuls (rhs = t/s, against the scaled weights).
The k-bias is skipped: softmax_j((Q0+bq).(K0+bk)) = softmax_j((Q0+bq).K0)
since the bk term only adds a per-row constant. The v-bias (sum of the
softmax weights times a constant = the constant) is folded through the
proj conv into the output bias.

Softmax runs without max-subtraction: logits here are ~N(0,1) after the
1/sqrt(C) scale, so exp stays comfortably finite in fp32.

Matmul dtype is bf16 with fp32 PSUM accumulation throughout (incl. Q@K
and A@V); the softmax normalizer, proj epilogue and residual are fp32.
"""

import numpy as np
import ml_dtypes

import concourse.bacc as bacc
import concourse.tile as tile
from concourse import mybir
from concourse import bass_utils

B, C, H, W = 2, 512, 64, 64
HW = H * W              # 4096 spatial positions
P = 128                 # partitions
KC = C // P             # 4 channel chunks
NCORES = 8
QB = B * HW // NCORES   # 1024 query positions per core
NIH = 2                 # query halves of 512
G = 8                   # groups
GSZ = C // G            # 64 channels / group
NPOS = GSZ * HW         # elements per group
NJT = HW // P           # 32 key tiles
EPS = 1e-6
SCALE = float(C) ** -0.5

F32 = mybir.dt.float32
BF16 = mybir.dt.bfloat16
AX = mybir.AxisListType
OP = mybir.AluOpType
AF = mybir.ActivationFunctionType


def _build(has_nw, has_nb, has_bq, has_bv, has_bp):
    nc = bacc.Bacc("TRN2", target_bir_lowering=False, debug=False,
                   num_devices=NCORES)

    xb_d = nc.dram_tensor("xb", [C, HW], BF16, kind="ExternalInput").ap()
    xq_d = nc.dram_tensor("xq", [C, QB], F32, kind="ExternalInput").ap()
    wt_d = nc.dram_tensor("wqkv", [3, C, C], F32, kind="ExternalInput").ap()
    wpt_d = nc.dram_tensor("wpt", [C, C], BF16, kind="ExternalInput").ap()
    ek_d = nc.dram_tensor("ek", [KC, P, G], F32, kind="ExternalInput").ap()
    ekb_d = nc.dram_tensor("ekb", [KC, P, G], BF16, kind="ExternalInput").ap()
    ones_d = nc.dram_tensor("ones32", [P, P], F32, kind="ExternalInput").ap()
    ekt_d = nc.dram_tensor("ekt", [KC, G, P], F32, kind="ExternalInput").ap()
    opt_d = {}
    for name, flag in (("nw", has_nw), ("nb", has_nb), ("bq", has_bq),
                       ("bv", has_bv), ("bp", has_bp)):
        if flag:
            opt_d[name] = nc.dram_tensor(
                name, [KC, P, 1], F32, kind="ExternalInput").ap()
    out_d = nc.dram_tensor("out", [C, QB], F32, kind="ExternalOutput").ap()

    with tile.TileContext(nc) as tc:
        _body(nc, tc, xb_d, xq_d, wt_d, wpt_d, ek_d, ekb_d, ekt_d,
              ones_d, opt_d, out_d, has_nw, has_nb, has_bq, has_bv, has_bp)

    nc.compile()
    return nc


def _body(nc, tc, xb_d, xq_d, wt_d, wpt_d, ek_d, ekb_d, ekt_d,
          ones_d, opt_d, out_d, has_nw, has_nb, has_bq, has_bv, has_bp):
    with (
        tc.tile_pool(name="kbuf", bufs=KC) as pk,
        tc.tile_pool(name="vt", bufs=1) as pvt,
        tc.tile_pool(name="qbuf", bufs=KC) as pq,
        tc.tile_pool(name="wp", bufs=1) as pwp,
        tc.tile_pool(name="xq", bufs=1) as pxq,
        tc.tile_pool(name="small", bufs=4) as ps,
    ):
        # ---- persistent tiles (packed; few big DMAs) -------------------
        k_bf = [pk.tile([P, HW], BF16, tag="kbuf", name=f"kbf{k}") for k in range(KC)]
        vt_bf = pvt.tile([P, NJT * C], BF16, name="vtbf")
        q_bf = [pq.tile([P, QB], BF16, tag="qbuf", name=f"qbf{k}") for k in range(KC)]

        wpt_b = pwp.tile([P, KC, C], BF16, name="wptb")
        nc.gpsimd.dma_start(out=wpt_b[:],
                            in_=wpt_d.rearrange("(k p) n -> p k n", p=P))
        wpt_t = [wpt_b[:, k, :] for k in range(KC)]
        xq_b = pxq.tile([P, KC, QB], F32, name="xqb32")
        nc.gpsimd.dma_start(out=xq_b[:],
                            in_=xq_d.rearrange("(k p) n -> p k n", p=P))
        xq_t = [xq_b[:, k, :] for k in range(KC)]
        ek_b = ps.tile([P, KC, G], F32, tag="ek", name="ekb")
        nc.gpsimd.dma_start(out=ek_b[:], in_=ek_d.rearrange("k p g -> p k g"))
        ek_t = [ek_b[:, k, :] for k in range(KC)]
        ekb_b = ps.tile([P, KC, G], BF16, tag="ekbf", name="ekbb")
        nc.gpsimd.dma_start(out=ekb_b[:], in_=ekb_d.rearrange("k p g -> p k g"))
        ekb_t = [ekb_b[:, k, :] for k in range(KC)]
        ones_t = ps.tile([P, P], F32, tag="ones", name="ones")
        nc.gpsimd.dma_start(out=ones_t[:], in_=ones_d[:])
        ekt_b = ps.tile([G, KC, P], F32, tag="ekt", name="ektb")
        nc.gpsimd.dma_start(out=ekt_b[:], in_=ekt_d.rearrange("k g p -> g k p"))
        ekt_t = [ekt_b[:, k, :] for k in range(KC)]
        opt_t = {}
        for name, ap in opt_d.items():
            ob = ps.tile([P, KC, 1], F32, tag=f"opt{name}", name=f"opt{name}b")
            nc.gpsimd.dma_start(out=ob[:], in_=ap.rearrange("k p o -> p k o"))
            opt_t[name] = [ob[:, k, :] for k in range(KC)]

        # per-channel scale (rstd*norm_w) and t/s (= -mean + norm_b/s)
        ch_t = [ps.tile([P, 2], F32, tag="ch", name=f"ch{k}") for k in range(KC)]
        scale_t = [ps.tile([P, 1], F32, tag="scale", name=f"scl{k}") for k in range(KC)]
        bos_t = [ps.tile([P, 1], BF16, tag="bos", name=f"bos{k}") for k in range(KC)]
        bqe_t = [ps.tile([P, 1], F32, tag="bqe", name=f"bqe{k}") for k in range(KC)]
        bve_t = [ps.tile([P, 1], BF16, tag="bve", name=f"bve{k}") for k in range(KC)]
        bpe_t = [ps.tile([P, 1], F32, tag="bpe", name=f"bpe{k}") for k in range(KC)]

        with (
            tc.tile_pool(name="xbuf", bufs=1) as px,
            tc.tile_pool(name="wf32", bufs=1) as pwf,
            tc.tile_pool(name="statps", bufs=1, space="PSUM") as pssm,
        ):
            x_b = px.tile([P, KC, HW], BF16, name="xbig")
            x_bf = [x_b[:, k, :] for k in range(KC)]
            xb_v = xb_d.rearrange("(k p) n -> p k n", p=P)
            NQT = 4
            QTR = HW // NQT
            for qt in range(NQT):
                sl = slice(QTR * qt, QTR * (qt + 1))
                nc.sync.dma_start(out=x_b[:, :, sl], in_=xb_v[:, :, sl])
            wf_b = pwf.tile([P, 3, KC, C], F32, name="wfb")
            nc.sync.dma_start(
                out=wf_b[:], in_=wt_d.rearrange("w (k p) n -> p w k n", p=P))
            wf_t = {w: [wf_b[:, wi, k, :] for k in range(KC)]
                    for wi, w in enumerate("qkv")}

            # ---- group stats (pipelined with the DMA) ------------------
            # s1 per group via indicator matmuls on PE (accumulating over
            # chunks AND position tiles into one [G, 512] psum), s2 via
            # x*x sum-reductions split across DVE and ACT.
            eps_t = ps.tile([G, 1], F32, tag="eps", name="eps")
            nc.gpsimd.memset(eps_t[:], float(EPS))
            warm = ps.tile([G, 1], F32, tag="warm", name="warm")
            nc.scalar.activation(out=warm[:], in_=eps_t[:], func=AF.Sqrt,
                                 bias=eps_t[:])
            nc.scalar.activation(out=warm[:], in_=eps_t[:], func=AF.Exp,
                                 scale=SCALE)

            s1ps = pssm.tile([G, 512], F32, tag="gps", name="s1ps")
            s2g = pssm.tile([G, 1], F32, tag="s2g", name="s2g")
            sqq_t = [ps.tile([P, NQT], F32, tag="sqq", name=f"sqq{k}")
                     for k in range(KC)]
            NT = HW // 512
            TPQ = NT // NQT
            idx = 0
            with tc.tile_pool(name="scratch", bufs=3) as psc:
                for qt in range(NQT):
                    for tt in range(TPQ):
                        t = qt * TPQ + tt
                        for k in range(KC):
                            nc.tensor.matmul(
                                s1ps[:], lhsT=ekb_t[k][:],
                                rhs=x_bf[k][:, 512 * t:512 * (t + 1)],
                                start=(idx == 0), stop=(idx == KC * NT - 1))
                            idx += 1
                    for k in range(KC):
                        sl = slice(QTR * qt, QTR * (qt + 1))
                        scr = psc.tile([P, QTR], BF16, tag="scr",
                                       name=f"scr{k}{qt}")
                        if (qt * KC + k) % 16 < 7:
                            nc.vector.tensor_tensor(
                                out=scr[:], in0=x_bf[k][:, sl],
                                in1=x_bf[k][:, sl], op=OP.mult)
                            nc.vector.tensor_reduce(
                                out=sqq_t[k][:, qt:qt + 1], in_=scr[:],
                                axis=AX.X, op=OP.add)
                        else:
                            nc.scalar.activation(
                                out=scr[:], in_=x_bf[k][:, sl],
                                func=AF.Square,
                                accum_out=sqq_t[k][:, qt:qt + 1])
                for k in range(KC):
                    s2ch = ps.tile([P, 1], F32, tag="s2ch", name=f"s2ch{k}")
                    nc.vector.tensor_reduce(
                        out=s2ch[:], in_=sqq_t[k][:], axis=AX.X, op=OP.add)
                    nc.tensor.matmul(s2g[:], lhsT=ek_t[k][:], rhs=s2ch[:],
                                     start=(k == 0), stop=(k == KC - 1))

            # mean/var/rstd per group
            gm = ps.tile([G, 2], F32, tag="gm", name="gm")
            nc.vector.tensor_reduce(
                out=gm[:, 0:1], in_=s1ps[:], axis=AX.X, op=OP.add)
            nc.vector.tensor_copy(out=gm[:, 1:2], in_=s2g[:])
            nc.vector.tensor_scalar_mul(gm[:], gm[:], 1.0 / NPOS)
            m2 = ps.tile([G, 1], F32, tag="m2", name="m2")
            nc.vector.tensor_tensor(
                out=m2[:], in0=gm[:, 0:1], in1=gm[:, 0:1], op=OP.mult)
            var = ps.tile([G, 1], F32, tag="var", name="var")
            nc.vector.tensor_tensor(
                out=var[:], in0=gm[:, 1:2], in1=m2[:], op=OP.subtract)
            std = ps.tile([G, 1], F32, tag="std", name="std")
            nc.scalar.activation(out=std[:], in_=var[:], func=AF.Sqrt,
                                 bias=eps_t[:])
            gb = ps.tile([G, 2], F32, tag="gb", name="gb")
            nc.vector.tensor_copy(out=gb[:, 0:1], in_=gm[:, 0:1])
            nc.vector.reciprocal(out=gb[:, 1:2], in_=std[:])

            # broadcast group stats back to channels
            for k in range(KC):
                bcp = pssm.tile([P, 2], F32, tag="bcp", name=f"bcp{k}")
                nc.tensor.matmul(bcp[:], lhsT=ekt_t[k][:], rhs=gb[:],
                                 start=True, stop=True)
                nc.vector.tensor_copy(out=ch_t[k][:], in_=bcp[:])
                if has_nw:
                    nc.vector.tensor_tensor(
                        out=scale_t[k][:], in0=ch_t[k][:, 1:2],
                        in1=opt_t["nw"][k][:], op=OP.mult)
                else:
                    nc.vector.tensor_copy(
                        out=scale_t[k][:], in_=ch_t[k][:, 1:2])
                # bos = t/s = -mean (+ norm_b / s)
                if has_nb:
                    rs = ps.tile([P, 1], F32, tag="rs", name=f"rs{k}")
                    nc.vector.reciprocal(out=rs[:], in_=scale_t[k][:])
                    nc.vector.tensor_tensor(
                        out=rs[:], in0=rs[:], in1=opt_t["nb"][k][:],
                        op=OP.mult)
                    nc.vector.scalar_tensor_tensor(
                        out=bos_t[k][:], in0=ch_t[k][:, 0:1], scalar=-1.0,
                        in1=rs[:], op0=OP.mult, op1=OP.add)
                else:
                    nc.vector.tensor_scalar_mul(
                        bos_t[k][:], ch_t[k][:, 0:1], -1.0)

            # ---- scaled weights + effective biases + convs -------------
            with (
                tc.tile_pool(name="wqkv", bufs=KC) as pw,
                tc.tile_pool(name="convps", bufs=4, space="PSUM") as pcv,
            ):
                ws = {}
                for wi, w in enumerate("qkv"):
                    ws[w] = [pw.tile([P, C], BF16, tag=f"w{w}", name=f"w{w}{k}")
                             for k in range(KC)]
                    for k in range(KC):
                        nc.scalar.activation(
                            out=ws[w][k][:], in_=wf_t[w][k][:], func=AF.Copy,
                            scale=scale_t[k][:])

                # K = wk_s.T @ x, laid out [cout, j] (no bias - cancels)
                for m in range(KC):
                    for t in range(HW // 512):
                        kp = pcv.tile([P, 512], F32, tag="cv", name=f"kp{m}{t}")
                        for k in range(KC):
                            nc.tensor.matmul(
                                kp[:],
                                lhsT=ws["k"][k][:, P * m:P * (m + 1)],
                                rhs=x_bf[k][:, 512 * t:512 * (t + 1)],
                                start=(k == 0), stop=(k == KC - 1))
                        nc.vector.tensor_copy(
                            out=k_bf[m][:, 512 * t:512 * (t + 1)], in_=kp[:])

                # VT = x.T @ wv_s, laid out [j, cout] in 32 j-tiles
                for jt in range(NJT):
                    vp = pcv.tile([P, 512], F32, tag="cv", name=f"vp{jt}")
                    for k in range(KC):
                        nc.tensor.matmul(
                            vp[:],
                            lhsT=x_bf[k][:, P * jt:P * (jt + 1)],
                            rhs=ws["v"][k][:],
                            start=(k == 0), stop=(k == KC - 1))
                    nc.vector.tensor_copy(
                        out=vt_bf[:, C * jt:C * (jt + 1)], in_=vp[:])

                # effective biases: beff_X[cout] = sum_cin wXs[cin,cout]*bos[cin]
                def beff(wtiles, dst, extra):
                    for m in range(KC):
                        bp_ps = pssm.tile([P, 1], F32, tag="beffps", name=f"bps{m}")
                        for k in range(KC):
                            nc.tensor.matmul(
                                bp_ps[:],
                                lhsT=wtiles[k][:, P * m:P * (m + 1)],
                                rhs=bos_t[k][:],
                                start=(k == 0), stop=(k == KC - 1))
                        if extra is not None:
                            nc.vector.tensor_tensor(
                                out=dst[m][:], in0=bp_ps[:],
                                in1=extra[m][:], op=OP.add)
                        else:
                            nc.vector.tensor_copy(out=dst[m][:], in_=bp_ps[:])

                beff(ws["q"], bqe_t, opt_t.get("bq"))
                beff(ws["v"], bve_t, opt_t.get("bv"))
                # fold v-bias through proj: bpe = wp @ bve (+ bp)
                for m in range(KC):
                    bp_ps = pssm.tile([P, 1], F32, tag="beffps", name=f"bpp{m}")
                    for k in range(KC):
                        nc.tensor.matmul(
                            bp_ps[:],
                            lhsT=wpt_t[k][:, P * m:P * (m + 1)],
                            rhs=bve_t[k][:],
                            start=(k == 0), stop=(k == KC - 1))
                    if has_bp:
                        nc.vector.tensor_tensor(
                            out=bpe_t[m][:], in0=bp_ps[:],
                            in1=opt_t["bp"][m][:], op=OP.add)
                    else:
                        nc.vector.tensor_copy(out=bpe_t[m][:], in_=bp_ps[:])

                # Q = wq_s.T @ xq (+bq_eff), laid out [cout, i]
                for m in range(KC):
                    for t in range(NIH):
                        qp = pcv.tile([P, 512], F32, tag="cv", name=f"qp{m}{t}")
                        for k in range(KC):
                            nc.tensor.matmul(
                                qp[:],
                                lhsT=ws["q"][k][:, P * m:P * (m + 1)],
                                rhs=x_bf[k][:, 512 * t:512 * (t + 1)],
                                start=(k == 0), stop=(k == KC - 1))
                        nc.vector.tensor_scalar_add(
                            q_bf[m][:, 512 * t:512 * (t + 1)],
                            qp[:], bqe_t[m][:])


        # ---- attention ---------------------------------------------
        with (
            tc.tile_pool(name="at", bufs=6) as pa,
            tc.tile_pool(name="obuf", bufs=2 * KC) as po,
            tc.tile_pool(name="rb", bufs=2) as prb,
            tc.tile_pool(name="outb", bufs=2) as pob,
            tc.tile_pool(name="acc", bufs=2) as pacc,
            tc.tile_pool(name="sps", bufs=3, space="PSUM") as psps,
            tc.tile_pool(name="ops", bufs=4, space="PSUM") as pops,
            tc.tile_pool(name="csps", bufs=1, space="PSUM") as pcs,
        ):
            for ih in range(NIH):
                i_sl = slice(512 * ih, 512 * (ih + 1))
                o_ps = [pops.tile([P, 512], F32, tag="ops", name=f"ops{m}")
                        for m in range(KC)]
                acc = pacc.tile([P, 512], F32, tag="acc", name=f"acc{ih}")
                ats = [None] * NJT

                LAG = 4

                def tail(jt):
                    # O[c] += VT[jt].T @ A
                    for m in range(KC):
                        nc.tensor.matmul(
                            o_ps[m][:],
                            lhsT=vt_bf[:, C * jt + P * m:C * jt + P * (m + 1)],
                            rhs=ats[jt][:],
                            start=(jt == 0), stop=(jt == NJT - 1))

                for jt in range(NJT):
                    sp = psps.tile([P, 512], F32, tag="sp", name=f"sp{jt}")
                    for k in range(KC):
                        nc.tensor.matmul(
                            sp[:],
                            lhsT=k_bf[k][:, P * jt:P * (jt + 1)],
                            rhs=q_bf[k][:, i_sl],
                            start=(k == 0), stop=(k == KC - 1))
                    at = pa.tile([P, 512], BF16, tag="at", name=f"at{jt}")
                    nc.scalar.activation(out=at[:], in_=sp[:], func=AF.Exp,
                                         scale=SCALE)
                    ats[jt] = at
                    if jt == 0:
                        nc.vector.tensor_copy(out=acc[:], in_=at[:])
                    else:
                        nc.vector.tensor_tensor(
                            out=acc[:], in0=acc[:], in1=at[:], op=OP.add)
                    if jt >= LAG:
                        tail(jt - LAG)
                for jt in range(NJT - LAG, NJT):
                    tail(jt)

                # normalize rows, then proj + residual
                cs_ps = pcs.tile([P, 512], F32, tag="cs", name=f"cs{ih}")
                nc.tensor.matmul(cs_ps[:], lhsT=ones_t[:], rhs=acc[:],
                                 start=True, stop=True)
                rb = prb.tile([P, 512], F32, tag="rb", name="rb")
                nc.vector.reciprocal_approx_fast(out=rb[:], in_=cs_ps[:])
                o_t = [po.tile([P, 512], BF16, tag="ob", name=f"ot{m}")
                       for m in range(KC)]
                for m in range(KC):
                    nc.vector.tensor_tensor(
                        out=o_t[m][:], in0=o_ps[m][:], in1=rb[:], op=OP.mult)
                ob = pob.tile([P, KC, 512], F32, tag="outb", name=f"outt{ih}")
                for m in range(KC):
                    pp = pops.tile([P, 512], F32, tag="ops", name=f"pp{m}")
                    for k in range(KC):
                        nc.tensor.matmul(
                            pp[:],
                            lhsT=wpt_t[k][:, P * m:P * (m + 1)],
                            rhs=o_t[k][:],
                            start=(k == 0), stop=(k == KC - 1))
                    nc.vector.scalar_tensor_tensor(
                        out=ob[:, m, :], in0=pp[:], scalar=bpe_t[m][:],
                        in1=xq_t[m][:, i_sl], op0=OP.add, op1=OP.add)
                out_v = out_d.rearrange("(k p) n -> p k n", p=P)
                nc.sync.dma_start(out=out_v[:, 0:2, i_sl], in_=ob[:, 0:2, :])
                nc.sync.dma_start(out=out_v[:, 2:4, i_sl], in_=ob[:, 2:4, :])


_NC_CACHE = {}


def _get_nc(flags):
    if flags not in _NC_CACHE:
        _NC_CACHE[flags] = _build(*flags)
    return _NC_CACHE[flags]


def _host_consts():
    ek = np.zeros((KC, P, G), np.float32)
    for k in range(KC):
        for p in range(P):
            ek[k, p, (p + P * k) // GSZ] = 1.0
    ekt = np.ascontiguousarray(ek.transpose(0, 2, 1))
    return ek, ekt


def prepare(inputs):
    x = np.ascontiguousarray(np.asarray(inputs["x"], np.float32))
    norm_w = np.asarray(inputs["norm_w"], np.float32)
    norm_b = np.asarray(inputs["norm_b"], np.float32)
    wts = {w: np.ascontiguousarray(
        np.asarray(inputs["w" + w], np.float32).T) for w in "qkvp"}
    bs = {w: np.asarray(inputs["b" + w], np.float32) for w in "qkvp"}
    wpt_bf = wts["p"].astype(ml_dtypes.bfloat16)
    wqkv = np.ascontiguousarray(np.stack([wts["q"], wts["k"], wts["v"]]))

    flags = (bool(np.any(norm_w != 1.0)), bool(np.any(norm_b != 0.0)),
             bool(np.any(bs["q"] != 0.0)), bool(np.any(bs["v"] != 0.0)),
             bool(np.any(bs["p"] != 0.0)))
    ek, ekt = _host_consts()
    in_maps = []
    for core in range(NCORES):
        b, qb = divmod(core, NCORES // B)
        xb = np.ascontiguousarray(x[b].reshape(C, HW))
        xq = np.ascontiguousarray(xb[:, qb * QB:(qb + 1) * QB])
        # keys permuted so this core's query block is first; softmax over the
        # key axis is permutation-invariant, queries/outputs stay in order
        xb_perm = np.concatenate(
            [xq, xb[:, :qb * QB], xb[:, (qb + 1) * QB:]], axis=1)
        m = {
            "xb": xb_perm.astype(ml_dtypes.bfloat16),
            "xq": xq,
            "wqkv": wqkv, "wpt": wpt_bf,
            "ek": ek, "ekb": ek.astype(ml_dtypes.bfloat16), "ekt": ekt,
            "ones32": np.ones((P, P), np.float32),
        }
        for name, flag, arr in (("nw", flags[0], norm_w), ("nb", flags[1], norm_b),
                                ("bq", flags[2], bs["q"]), ("bv", flags[3], bs["v"]),
                                ("bp", flags[4], bs["p"])):
            if flag:
                m[name] = np.ascontiguousarray(arr.reshape(KC, P, 1))
        in_maps.append(m)
    return flags, in_maps


def assemble(results):
    out = np.empty((B, C, HW), np.float32)
    for core in range(NCORES):
        b, qb = divmod(core, NCORES // B)
        out[b][:, qb * QB:(qb + 1) * QB] = results[core]["out"]
    return out.reshape(B, C, H, W)


def run(inputs, **spmd_kwargs):
    flags, in_maps = prepare(inputs)
    nc = _get_nc(flags)
    res = bass_utils.run_bass_kernel_spmd(nc, in_maps, list(range(NCORES)),
                                          **spmd_kwargs)
    return assemble(res.results), res


def kernel(**inputs):
    out, _ = run(inputs)
    return out
